# revision 21
# baseline (speedup 1.0000x reference)
"""Additive (Bahdanau) attention on 8 Trainium2 NeuronCores.

Problem shapes (hardcoded): query [2,1024,256], key [2,1024,256],
Wa_w/Wb_w [256,128], Wa_b/Wb_b [128], v_w [128].  Output [2,1024,256].

  a = q @ Wa + Wa_b                  [B,N,H]
  b = k @ Wb + Wb_b                  [B,M,H]
  s[b,n,m] = sum_h v_h tanh(a[b,n,h] + b[b,m,h])
  out = softmax_m(s) @ key           [B,N,D]

Sharding: 8 cores = B(2) x N-blocks(4).  Each core: 256 queries, full key.

Per-core algorithm (H=128 lives on SBUF partitions):
  aT[h,n]  = (Wa^T q^T)        via PE, no bias
  bTc[h,m] = (Wb^T k^T) + (Wa_b+Wb_b)  combined bias folded here
  per query n: arg[h,m] = bTc[h,m] + aT[h,n]   (DVE tensor_scalar, fp16 4x)
  tanh on ACT in [128, 16*1024] mega-tiles (16 queries per instruction);
  ACT runs tanh at 1 elem/lane/cycle -> ~218us/core is the roofline and
  the kernel sits on it (everything else is hidden behind ACT)
  scores block [128n, 1024m] accumulated in PSUM by 256 "delta-weight"
  matmuls: lhsT = Z[:, 127-j:255-j] where Z has v at column 127 and zeros
  elsewhere -> writes v.tanh contraction into score row j only.
  exp+rowsum fused on ACT (accum_out), reciprocal + scale on DVE,
  PE-transpose attn -> attnT[m,n], final PE matmul key^T-accumulate
  -> outT [d, n] -> DRAM.  Host transposes back.
"""

import numpy as np

import concourse.bass as bass
import concourse.tile as tile
from concourse import bacc, mybir
from concourse import bass_utils
from concourse.masks import make_identity

F32 = mybir.dt.float32

B, N, M, D, H = 2, 1024, 1024, 256, 128
NCORES = 8
NBLK = 4          # n-blocks per batch entry
NCORE = N // NBLK  # 256 queries per core
CHUNK = 8          # queries per ACT tanh instruction


def build_nc(reps: int = 1, **opts):
    nc = bacc.Bacc(
        "TRN2",
        target_bir_lowering=False,
        debug=False,
        enable_asserts=False,
        num_devices=NCORES,
    )
    qT_d = nc.dram_tensor("qT", [D, NCORE], F32, kind="ExternalInput").ap()
    k_d = nc.dram_tensor("k", [M, D], F32, kind="ExternalInput").ap()
    kT_d = nc.dram_tensor("kT", [D, M], F32, kind="ExternalInput").ap()
    wa_d = nc.dram_tensor("wa", [D, H], F32, kind="ExternalInput").ap()
    wb_d = nc.dram_tensor("wb", [D, H], F32, kind="ExternalInput").ap()
    bias_d = nc.dram_tensor("bias", [H, 1], F32, kind="ExternalInput").ap()
    v_d = nc.dram_tensor("v", [H, 1], F32, kind="ExternalInput").ap()
    out_d = nc.dram_tensor("out", [D, NCORE], F32, kind="ExternalOutput").ap()

    with tile.TileContext(nc) as tc:
        _build_body(tc, qT_d, k_d, kT_d, wa_d, wb_d, bias_d, v_d, out_d, reps,
                    **opts)
    nc.compile()
    return nc


def _build_body(tc, qT_d, k_d, kT_d, wa_d, wb_d, bias_d, v_d, out_d, reps,
                f16_args=False, f16_th=False, skip_tanh=False,
                stop_after_tanh=False, stop_after_scores=False, wbufs=2,
                chunk=CHUNK, sc_bufs=1, act_bias=False, fuse_every=0,
                act_copies=False, no_args=False, tanh_dma=False,
                args_prio=0):
    nc = tc.nc
    KT = D // 128  # 2 contraction tiles over d
    ADT = mybir.dt.float16 if f16_args else F32
    TDT = mybir.dt.float16 if f16_th else F32

    with (
        tc.tile_pool(name="persist", bufs=1) as pp,
        tc.tile_pool(name="work", bufs=wbufs) as wp,
        tc.tile_pool(name="small", bufs=4) as sp,
    ):
        # ---- static loads (order matters for startup latency: weights and
        # kT feed the projections that gate everything; k_nat is only needed
        # by the last matmul stage, load it last) ----
        kT_sb = []
        qT_sb = []
        wa_sb = []
        wb_sb = []
        for dt_ in range(KT):
            watile = pp.tile([128, H], F32, name=f"wa{dt_}")
            nc.sync.dma_start(watile[:], wa_d[dt_ * 128:(dt_ + 1) * 128, :])
            wa_sb.append(watile)
            wbtile = pp.tile([128, H], F32, name=f"wb{dt_}")
            nc.sync.dma_start(wbtile[:], wb_d[dt_ * 128:(dt_ + 1) * 128, :])
            wb_sb.append(wbtile)
            qtile = pp.tile([128, NCORE], F32, name=f"qT{dt_}")
            nc.sync.dma_start(qtile[:], qT_d[dt_ * 128:(dt_ + 1) * 128, :])
            qT_sb.append(qtile)
        bias_sb = pp.tile([128, 1], F32, name="bias_sb")
        nc.sync.dma_start(bias_sb[:], bias_d[:, :])
        v_sb = pp.tile([128, 1], F32, name="v_sb")
        nc.sync.dma_start(v_sb[:], v_d[:, :])
        for dt_ in range(KT):
            ktile = pp.tile([128, M], F32, name=f"kT{dt_}")
            # split per 512-col chunk: subtile deps let the b-projection
            # matmul for chunk mc start as soon as its halves land
            for mc in range(2):
                nc.sync.dma_start(
                    ktile[:, mc * 512:(mc + 1) * 512],
                    kT_d[dt_ * 128:(dt_ + 1) * 128, mc * 512:(mc + 1) * 512])
            kT_sb.append(ktile)
        k_nat = []
        for mt in range(M // 128):
            kt = pp.tile([128, D], F32, name=f"k_nat{mt}")
            nc.sync.dma_start(kt[:], k_d[mt * 128:(mt + 1) * 128, :])
            k_nat.append(kt)

        # delta-weight tensor: zeros with v at column 127
        zv = pp.tile([128, 256], TDT, name="zv")
        nc.gpsimd.memset(zv[:], 0.0)
        nc.vector.tensor_copy(zv[:, 127:128], v_sb[:])

        ident = pp.tile([128, 128], F32, name="ident")
        make_identity(nc, ident[:])

        aT = pp.tile([128, NCORE], F32, name="aT")
        bTc = pp.tile([128, M], ADT, name="bTc")

        tr_bufs = 1 if sc_bufs == 2 else 2
        with (
            tc.tile_pool(name="proj_ps", bufs=1, space="PSUM") as projp,
            tc.tile_pool(name="sc_ps", bufs=sc_bufs, space="PSUM") as scp,
            tc.tile_pool(name="tr_ps", bufs=tr_bufs, space="PSUM") as trp,
            tc.tile_pool(name="o_ps", bufs=tr_bufs, space="PSUM") as opp,
        ):
            for _ in range(reps):
                # ---- projections ----
                ps_a = projp.tile([128, NCORE], F32, name="ps_a")
                for dt_ in range(KT):
                    nc.tensor.matmul(
                        ps_a[:], wa_sb[dt_][:], qT_sb[dt_][:],
                        start=(dt_ == 0), stop=(dt_ == KT - 1),
                    )
                nc.vector.tensor_copy(aT[:], ps_a[:])
                for mc in range(2):
                    ps_b = projp.tile([128, 512], F32, name="ps_b")
                    for dt_ in range(KT):
                        nc.tensor.matmul(
                            ps_b[:], wb_sb[dt_][:],
                            kT_sb[dt_][:, mc * 512:(mc + 1) * 512],
                            start=(dt_ == 0), stop=(dt_ == KT - 1),
                        )
                    nc.vector.tensor_scalar_add(
                        bTc[:, mc * 512:(mc + 1) * 512], ps_b[:], bias_sb[:]
                    )

                # ---- main loop over two 128-query blocks ----
                for nb in range(2):
                    sc = scp.tile([128, M], F32, name="sc")
                    nchunks = 128 // chunk
                    for ch in range(nchunks):
                        if act_bias:
                            th = wp.tile([128, chunk * M], TDT, name="th")
                            for i in range(chunk):
                                n = nb * 128 + ch * chunk + i
                                nc.scalar.activation(
                                    th[:, i * M:(i + 1) * M], bTc[:],
                                    mybir.ActivationFunctionType.Tanh,
                                    bias=aT[:, n:n + 1],
                                )
                        elif True:
                            arg = wp.tile([128, chunk * M], ADT, name="arg")
                            fused = [
                                i for i in range(chunk)
                                if fuse_every and i % fuse_every == fuse_every - 1
                            ]
                            th = wp.tile([128, chunk * M], TDT, name="th") \
                                if not skip_tanh else arg
                            import contextlib
                            prio = tc.high_priority(offset=args_prio) \
                                if args_prio else contextlib.nullcontext()
                            with prio:
                                for i in range(chunk):
                                    n = nb * 128 + ch * chunk + i
                                    if i in fused or no_args:
                                        continue
                                    nc.vector.tensor_scalar_add(
                                        arg[:, i * M:(i + 1) * M], bTc[:],
                                        aT[:, n:n + 1]
                                    )
                            if not skip_tanh:
                                # contiguous runs of non-fused i -> one big tanh
                                run = []
                                for i in range(chunk + 1):
                                    if i < chunk and i not in fused:
                                        run.append(i)
                                        continue
                                    if run:
                                        lo, hi = run[0], run[-1] + 1
                                        nc.scalar.activation(
                                            th[:, lo * M:hi * M],
                                            arg[:, lo * M:hi * M],
                                            mybir.ActivationFunctionType.Tanh,
                                        )
                                        run = []
                                    if i < chunk:
                                        n = nb * 128 + ch * chunk + i
                                        nc.scalar.activation(
                                            th[:, i * M:(i + 1) * M], bTc[:],
                                            mybir.ActivationFunctionType.Tanh,
                                            bias=aT[:, n:n + 1],
                                        )
                        if tanh_dma:
                            w = NCORE // (128 // chunk)
                            nc.sync.dma_start(
                                out_d[nb * 128:(nb + 1) * 128,
                                      ch * w:(ch + 1) * w],
                                th[:, :2 * w].bitcast(F32)
                                if th.dtype != F32 else th[:, :w])
                            continue
                        if stop_after_tanh:
                            if ch == 0:
                                dump = th[:, :512].bitcast(F32) \
                                    if th.dtype != F32 else th[:, :NCORE]
                                nc.sync.dma_start(out_d[0:128, :], dump)
                                nc.sync.dma_start(out_d[128:256, :], dump)
                            continue
                        for i in range(chunk):
                            j = ch * chunk + i  # row within block
                            for mc in range(2):
                                nc.tensor.matmul(
                                    sc[:, mc * 512:(mc + 1) * 512],
                                    zv[:, 127 - j:255 - j],
                                    th[:, i * M + mc * 512:i * M + (mc + 1) * 512],
                                    start=(j == 0),
                                    stop=(j == 127),
                                )
                    if stop_after_tanh or tanh_dma:
                        continue
                    # softmax over m (free axis), no max-shift needed:
                    # |scores| <= sum|v| ~ 9 so exp is safe in fp32
                    ex = wp.tile([128, M], F32, name="ex")
                    sums = sp.tile([128, 1], F32, name="sums")
                    nc.scalar.activation(
                        ex[:], sc[:], mybir.ActivationFunctionType.Exp,
                        accum_out=sums[:],
                    )
                    if stop_after_scores:
                        nc.sync.dma_start(
                            out_d[0:128, nb * 128:(nb + 1) * 128],
                            ex[:, :128])
                        nc.sync.dma_start(
                            out_d[128:256, nb * 128:(nb + 1) * 128],
                            ex[:, 128:256])
                        continue
                    rs = sp.tile([128, 1], F32, name="rs")
                    nc.vector.reciprocal(rs[:], sums[:])
                    at = wp.tile([128, M], F32, name="at")
                    if act_copies:
                        nc.scalar.activation(
                            at[:], ex[:], mybir.ActivationFunctionType.Copy,
                            scale=rs[:])
                    else:
                        nc.vector.tensor_scalar_mul(at[:], ex[:], rs[:])

                    # transpose attn -> attnT [m, n] tiles
                    atT = wp.tile([128, M // 128, 128], F32, name="atT")
                    for mt in range(M // 128):
                        tp = trp.tile([128, 128], F32, name="tp")
                        nc.tensor.transpose(
                            tp[:], at[:, mt * 128:(mt + 1) * 128], ident[:]
                        )
                        if act_copies:
                            nc.scalar.copy(atT[:, mt, :], tp[:])
                        else:
                            nc.vector.tensor_copy(atT[:, mt, :], tp[:])

                    # outT[d, n] = sum_m key[m, d] attnT[m, n]
                    for dc in range(2):
                        ops = opp.tile([128, 128], F32, name="ops")
                        for mt in range(M // 128):
                            nc.tensor.matmul(
                                ops[:],
                                k_nat[mt][:, dc * 128:(dc + 1) * 128],
                                atT[:, mt, :],
                                start=(mt == 0), stop=(mt == M // 128 - 1),
                            )
                        osb = sp.tile([128, 128], F32, name="osb")
                        nc.vector.tensor_copy(osb[:], ops[:])
                        nc.sync.dma_start(
                            out_d[dc * 128:(dc + 1) * 128,
                                  nb * 128:(nb + 1) * 128],
                            osb[:],
                        )


def _in_maps(inputs):
    q = np.asarray(inputs["query"], dtype=np.float32)
    k = np.asarray(inputs["key"], dtype=np.float32)
    wa = np.ascontiguousarray(np.asarray(inputs["Wa_w"], dtype=np.float32))
    wb = np.ascontiguousarray(np.asarray(inputs["Wb_w"], dtype=np.float32))
    bias = (np.asarray(inputs["Wa_b"], dtype=np.float32)
            + np.asarray(inputs["Wb_b"], dtype=np.float32)).reshape(H, 1)
    v = np.asarray(inputs["v_w"], dtype=np.float32).reshape(H, 1)
    maps = []
    for c in range(NCORES):
        b, nblk = divmod(c, NBLK)
        n0 = nblk * NCORE
        maps.append({
            "qT": np.ascontiguousarray(q[b, n0:n0 + NCORE, :].T),
            "k": np.ascontiguousarray(k[b]),
            "kT": np.ascontiguousarray(k[b].T),
            "wa": wa,
            "wb": wb,
            "bias": bias,
            "v": v,
        })
    return maps


def _gather(results):
    out = np.empty((B, N, D), dtype=np.float32)
    for c in range(NCORES):
        b, nblk = divmod(c, NBLK)
        n0 = nblk * NCORE
        out[b, n0:n0 + NCORE, :] = results[c]["out"].T
    return out


_NC_CACHE = {}

# fp16 intermediates (args + tanh values); fp32 accumulation in PSUM.
# Measured output rel err ~2.8e-4 (scale-relative), dominated by fp16
# rounding of the tanh argument; fp32 variant available via BEST_OPTS={}.
BEST_OPTS = dict(f16_args=True, f16_th=True, chunk=16)


def _get_nc(reps=1):
    if reps not in _NC_CACHE:
        _NC_CACHE[reps] = build_nc(reps, **BEST_OPTS)
    return _NC_CACHE[reps]


def kernel(**inputs):
    nc = _get_nc(1)
    res = bass_utils.run_bass_kernel_spmd(
        nc, _in_maps(inputs), core_ids=list(range(NCORES))
    )
    return _gather(res.results)
